# revision 8
# baseline (speedup 1.0000x reference)
"""Trainium2 Bass kernel for nn_MultiHeadFast (multi-head attention with
softmax over the QUERY axis).

Math (faithful to the reference):
  qkv = x @ Ws;  per (b,h):  S[q,k] = Q.K^T,  causal mask k<=q,
  P = softmax_over_q(S * T^-0.5),  out = P @ V.

Layout strategy (v4):
  * Host passes x^T pre-transposed + pre-cast to bf16 as [128 e-partitions,
    8 e-blocks, 4096 tokens] plus the per-core Ws column slice in matching
    layout — no on-device transposes at all.
  * Q^T/K^T (features on partitions) from matmuls with Ws as lhsT; V is
    computed token-natural by swapping operands.  Output is written
    transposed; the host un-transposes at gather.
  * S is computed TRANSPOSED (S^T[k,q], keys on partitions): the q-axis
    softmax is a free-axis reduction and S^T feeds out^T = V^T @ P.  The
    per-key normalizer folds into V rows.  Strips are TIGHT: strip for key
    tile k covers q in [128k, T) exactly (first-slab PV matmuls are
    partial-width), so no dead columns anywhere.
  * Schedule: batch-0 QKV, then a merged loop interleaving batch-0
    attention units (S+exp+PV) with batch-1 S/exp units (strips + scaled-V
    persist in SBUF) and batch-1 QKV injections — keeping the ACT exp
    stream (the hard ~100us floor) continuously fed while the PE chews
    through projection work.  Batch-1's PV matmuls run in a PE-only tail
    phase on the PSUM banks batch-0 vacated.
  * PSUM: 2x[128,1024] shared ring + 4 PV banks = exactly 8 banks.

Sharding: tensor-parallel over heads.  Core c owns heads {2c, 2c+1}; no
collectives.  bf16 with fp32 accumulation (~4e-3 L2 vs fp32 reference).
"""

import numpy as np
import ml_dtypes
from contextlib import ExitStack

import concourse.bass as bass
import concourse.mybir as mybir
import concourse.tile as tile
from concourse import bacc
from concourse.bass_utils import run_bass_kernel_spmd

B, T, E = 2, 2048, 1024
H, D = 16, 64
NCORES = 8
HPC = H // NCORES            # heads per core = 2
FPC = HPC * D                # feature cols per core per Q/K/V = 128
P = 128
NT = B * T                   # 4096 tokens total
EK = E // P                  # 8 contraction blocks for QKV
NSLAB = T // 512             # 4 query slabs per batch
KTILES = T // P              # 16 key tiles per batch
DT = mybir.dt.bfloat16
F32 = mybir.dt.float32
BF16 = ml_dtypes.bfloat16
SCALE = float(T) ** -0.5
NEG = -1e30


def build_kernel():
    nc = bacc.Bacc("TRN2", target_bir_lowering=False, debug=False)
    xt_dram = nc.dram_tensor("xt", (P, EK, NT), DT, kind="ExternalInput")
    w_dram = nc.dram_tensor("wsl", (P, EK, 3 * FPC), DT, kind="ExternalInput")
    out_dram = nc.dram_tensor("out", (B, FPC, T), F32, kind="ExternalOutput")

    with tile.TileContext(nc) as tc, ExitStack() as ctx:
        const = ctx.enter_context(tc.tile_pool(name="const", bufs=1))
        xtp = ctx.enter_context(tc.tile_pool(name="xtp", bufs=1))
        qkvp = ctx.enter_context(tc.tile_pool(name="qkvp", bufs=1))
        strips = ctx.enter_context(tc.tile_pool(name="strips", bufs=4))
        keep = ctx.enter_context(tc.tile_pool(name="keep", bufs=1))
        small = ctx.enter_context(tc.tile_pool(name="small", bufs=8))
        outp = ctx.enter_context(tc.tile_pool(name="outp", bufs=2))
        ps = ctx.enter_context(tc.tile_pool(name="ps", bufs=2, space="PSUM"))

        # ---- constants ----
        zeros_bf = const.tile([P, P], DT, name="zeros_bf")
        nc.gpsimd.memset(zeros_bf[:], 0.0)
        # diagmask[p, f] = 0 if f >= p else NEG   (keys on partitions, q free)
        diagmask = const.tile([P, P], F32, name="diagmask")
        nc.gpsimd.memset(diagmask[:], 0.0)
        nc.gpsimd.affine_select(
            out=diagmask[:],
            in_=diagmask[:],
            compare_op=mybir.AluOpType.is_ge,
            fill=NEG,
            base=0,
            pattern=[[1, P]],
            channel_multiplier=-1,
        )

        # ---- inputs (already bf16 + transposed on host) ----
        wsl = qkvp.tile([P, EK, 3 * FPC], DT, name="wsl")
        nc.sync.dma_start(wsl[:], w_dram[:])
        xT = xtp.tile([P, EK, NT], DT, name="xT")
        for s in range(NT // 512):
            nc.sync.dma_start(
                xT[:, :, 512 * s : 512 * (s + 1)],
                xt_dram[:, :, 512 * s : 512 * (s + 1)],
            )

        qt = qkvp.tile([P, NT], DT, name="qt")
        kt = qkvp.tile([P, NT], DT, name="kt")
        v_nat = qkvp.tile([P, B * KTILES, FPC], DT, name="v_nat")

        # persistent batch-1 strips and scaled-V tiles (PV deferred to tail)
        strips_b1 = {
            (k, hh): keep.tile([P, T - P * k], DT, name=f"sb1_{k}_{hh}")
            for k in range(KTILES)
            for hh in range(HPC)
        }
        vp_b1 = {
            (k, hh): keep.tile([P, D], DT, name=f"vpb1_{k}_{hh}")
            for k in range(KTILES)
            for hh in range(HPC)
        }

        def ring(name):
            return ps.tile([P, 1024], F32, tag="ring", bufs=2, name=name)

        def qk_group(b, c, m):
            """Q (m=0) or K (m=1) projection for one 512-token chunk."""
            t0 = b * T + 512 * c
            slot = ring(f"qk{b}{c}{m}")
            for e in range(EK):
                nc.tensor.matmul(
                    slot[:, 0:512],
                    lhsT=wsl[:, e, m * FPC : (m + 1) * FPC],
                    rhs=xT[:, e, t0 : t0 + 512],
                    start=(e == 0),
                    stop=(e == EK - 1),
                )
            dst = qt if m == 0 else kt
            nc.vector.tensor_copy(dst[:, t0 : t0 + 512], slot[:, 0:512])

        def v_group(b, w):
            """V projection, token-natural, for one 128-token tile."""
            t0 = b * T + P * w
            slot = ring(f"v{b}{w}")
            for e in range(EK):
                nc.tensor.matmul(
                    slot[:, 0:FPC],
                    lhsT=xT[:, e, t0 : t0 + P],
                    rhs=wsl[:, e, 2 * FPC : 3 * FPC],
                    start=(e == 0),
                    stop=(e == EK - 1),
                )
            nc.vector.tensor_copy(v_nat[:, b * KTILES + w, :], slot[:, 0:FPC])

        pv_banks = {}   # b -> [4 psum tiles]
        sv_prev = {}    # b -> {hh: (strip, vp)} awaiting PV (batch 0 only)

        def pv_init(b):
            banks = [
                ps.tile([P, 512], F32, tag="pvb", bufs=4, name=f"pv{b}{j}")
                for j in range(NSLAB)
            ]
            for j in range(NSLAB):
                nc.tensor.matmul(
                    banks[j][:],
                    lhsT=zeros_bf[:],
                    rhs=xT[:, 0, 0:512],
                    start=True,
                    stop=False,
                    skip_group_check=True,
                )
            pv_banks[b] = banks

        def s_unit(b, k, strip_k):
            """S^T matmuls + mask + exp + normalizer for key tile k of batch
            b, both heads.  Tight strips: strip col x <-> q = 128k + x."""
            j0 = k // 4
            q0 = 512 * j0
            dead = P * k - q0
            L = T - q0
            parts = {0: [], 1: []}
            coff = 0
            while coff < L:
                cw = min(1024, L - coff)
                sps = {hh: ring(f"sps{b}{k}{hh}{coff}") for hh in range(HPC)}
                lo = dead if coff == 0 else 0
                for s0 in range(0, cw, 512):
                    a = max(s0, lo)
                    hi = min(s0 + 512, cw)
                    if a >= hi:
                        continue
                    qs = b * T + q0 + coff + a
                    for hh in range(HPC):
                        nc.tensor.matmul(
                            sps[hh][:, a:hi],
                            lhsT=kt[
                                hh * D : (hh + 1) * D,
                                b * T + k * P : b * T + k * P + P,
                            ],
                            rhs=qt[hh * D : (hh + 1) * D, qs : qs + (hi - a)],
                            start=True,
                            stop=True,
                        )
                for hh in range(HPC):
                    acc = small.tile([P, 1], F32, tag="acc", name="acc")
                    if coff == 0:
                        nc.vector.tensor_add(
                            sps[hh][:, dead : dead + P],
                            sps[hh][:, dead : dead + P],
                            diagmask[:],
                        )
                        nc.scalar.activation(
                            strip_k[hh][:, 0 : cw - dead],
                            sps[hh][:, dead:cw],
                            mybir.ActivationFunctionType.Exp,
                            scale=SCALE,
                            accum_out=acc[:],
                        )
                    else:
                        nc.scalar.activation(
                            strip_k[hh][:, coff - dead : coff - dead + cw],
                            sps[hh][:, :cw],
                            mybir.ActivationFunctionType.Exp,
                            scale=SCALE,
                            accum_out=acc[:],
                        )
                    parts[hh].append(acc)
                coff += cw
            sv = {}
            for hh in range(HPC):
                if len(parts[hh]) == 1:
                    ssum = parts[hh][0]
                else:
                    ssum = small.tile([P, 1], F32, tag="acc", name="ssum")
                    nc.vector.tensor_add(
                        ssum[:], parts[hh][0][:], parts[hh][1][:]
                    )
                rsum = small.tile([P, 1], F32, tag="acc", name="rsum")
                nc.vector.reciprocal(rsum[:], ssum[:])
                if b == 1:
                    vp = vp_b1[(k, hh)]
                else:
                    vp = small.tile([P, D], DT, tag="vp", name="vp")
                nc.vector.tensor_scalar_mul(
                    vp[:], v_nat[:, b * KTILES + k, hh * D : (hh + 1) * D], rsum[:]
                )
                sv[hh] = (strip_k[hh], vp)
            return sv

        def pv_unit(b, k, sv):
            """PV matmuls for key tile k, both heads.  Tight strips: the
            first slab's matmul is partial-width (masked columns stay at the
            bank's zero-init value, which is correct)."""
            j0 = k // 4
            dead = P * k - 512 * j0
            for j in range(j0, NSLAB):
                for hh in range(HPC):
                    strip, vp = sv[hh]
                    if j == j0 and dead > 0:
                        o, w = dead, 512 - dead
                        soff = 0
                    else:
                        o, w = 0, 512
                        soff = 512 * j - P * k
                    nc.tensor.matmul(
                        pv_banks[b][j][hh * D : (hh + 1) * D, o : o + w],
                        lhsT=vp[:],
                        rhs=strip[:, soff : soff + w],
                        start=False,
                        stop=(k == 4 * j + 3 and hh == HPC - 1),
                        skip_group_check=True,
                    )

        def evac(b, j):
            osb = outp.tile([P, 512], F32, tag="osb", bufs=2, name=f"osb{b}{j}")
            nc.vector.tensor_copy(osb[:], pv_banks[b][j][:])
            nc.sync.dma_start(out_dram[b, :, 512 * j : 512 * (j + 1)], osb[:])

        def b0_unit(k):
            strip_k = {
                hh: strips.tile(
                    [P, T - P * k], DT, tag="strip", name=f"s{k}{hh}"
                )
                for hh in range(HPC)
            }
            sv = s_unit(0, k, strip_k)
            if k > 0:
                pv_unit(0, k - 1, sv_prev[0])
            sv_prev[0] = sv
            if k >= 4 and k % 4 == 0:
                evac(0, k // 4 - 1)

        def b1_unit(k):
            strip_k = {hh: strips_b1[(k, hh)] for hh in range(HPC)}
            s_unit(1, k, strip_k)

        # ================= emission schedule =================
        # A0: batch-0 Q (all), K chunk 0, first V tiles; then pv init.
        for c in range(4):
            qk_group(0, c, 0)
        qk_group(0, 0, 1)
        v_group(0, 0)
        v_group(0, 1)
        pv_init(0)

        # Combined slots: b0 units 0..8 at slots 0..8 (b1 not ready yet),
        # then alternate b0/b1 units; remaining b1 units at the end.
        slots = []
        for k in range(9):
            slots.append(("b0", k))
        bi = 0
        for k in range(9, KTILES):
            slots.append(("b1", bi)); bi += 1
            slots.append(("b0", k))
        while bi < KTILES:
            slots.append(("b1", bi)); bi += 1

        # injections keyed by slot index
        inj = {i: [] for i in range(len(slots))}
        for c in range(1, 4):       # batch-0 K chunks 1..3
            inj[c - 1].append(lambda c=c: qk_group(0, c, 1))
        for g in range(8):          # batch-1 Q/K
            c, m = divmod(g, 2)
            inj[g].append(lambda c=c, m=m: qk_group(1, c, m))
        for t in range(2, KTILES):  # batch-0 V tiles, two slots ahead
            inj[t - 2].append(lambda t=t: v_group(0, t))
        # batch-1 V tiles: tile t must precede its b1 unit (slot 9+2t for
        # t<=6, slot 16+t for t>=7), with ~2 slots of lead.
        for t in range(KTILES):
            target = 9 + 2 * t if t <= 6 else 16 + t
            inj[min(target - 2, len(slots) - 1)].append(
                lambda t=t: v_group(1, t)
            )

        for i, (which, k) in enumerate(slots):
            if which == "b0":
                b0_unit(k)
            else:
                b1_unit(k)
            for fn in inj[i]:
                fn()
        pv_unit(0, KTILES - 1, sv_prev[0])
        evac(0, NSLAB - 1)

        # ============ phase 2: batch-1 PV (PE-only tail) ============
        pv_init(1)
        for k in range(KTILES):
            pv_unit(
                1, k, {hh: (strips_b1[(k, hh)], vp_b1[(k, hh)]) for hh in range(HPC)}
            )
            if k == 3:
                evac(1, 0)
            elif k == 7:
                evac(1, 1)
            elif k == 11:
                evac(1, 2)
        evac(1, 3)

    nc.compile()
    return nc


def make_in_maps(x: np.ndarray, Ws: np.ndarray):
    """Host-side shard + layout prep: x^T and per-core Ws slices, bf16,
    laid out [128 e-partitions, 8 e-blocks, cols] to match the kernel."""
    x2 = x.reshape(NT, E)
    xt = (
        np.ascontiguousarray(x2.T.reshape(EK, P, NT).transpose(1, 0, 2))
        .astype(BF16)
    )
    in_maps = []
    for c in range(NCORES):
        cols = np.concatenate(
            [
                Ws[:, c * FPC : (c + 1) * FPC],
                Ws[:, E + c * FPC : E + (c + 1) * FPC],
                Ws[:, 2 * E + c * FPC : 2 * E + (c + 1) * FPC],
            ],
            axis=1,
        )
        wsl = np.ascontiguousarray(
            cols.reshape(EK, P, 3 * FPC).transpose(1, 0, 2)
        ).astype(BF16)
        in_maps.append({"xt": xt, "wsl": wsl})
    return in_maps


def gather_out(results) -> np.ndarray:
    """Assemble per-core transposed outputs into the full (B, T, H*D)."""
    out = np.empty((B, T, H * D), np.float32)
    for c in range(NCORES):
        out[:, :, c * FPC : (c + 1) * FPC] = results[c]["out"].transpose(0, 2, 1)
    return out


_NC_CACHE = None


def kernel(x: np.ndarray, Ws: np.ndarray) -> np.ndarray:
    global _NC_CACHE
    if _NC_CACHE is None:
        _NC_CACHE = build_kernel()
    nc = _NC_CACHE

    in_maps = make_in_maps(
        np.asarray(x, np.float32), np.asarray(Ws, np.float32)
    )
    res = run_bass_kernel_spmd(nc, in_maps, core_ids=list(range(NCORES)))
    return gather_out(res.results)
